# revision 1
# baseline (speedup 1.0000x reference)
"""Binary (sign-quantized weight) 3x3 conv, stride 1, pad 1, on 8 trn2 cores.

Problem: x[32,128,56,56] f32, weight[256,128,3,3] f32, bias[256] f32
         y = conv2d(x, sign(weight), pad=1) + bias      -> [32,256,56,56] f32

Strategy:
  - Data-parallel over batch: 4 images per core, weight/bias replicated.
  - Per core: x is loaded per-image as [ci=128 partitions, 56*56], cast to
    bf16 into a zero-padded [128, 58*58] tile. The 3x3 conv is 9 shifted
    [128ci -> 128co] matmuls accumulated in PSUM (implicit im2col via
    strided access patterns into the padded tile).
  - Weights are host-relaid to [ci, (kh kw co)] f32; sign+cast to bf16 on
    device (ScalarE). bf16 is exact for {-1,0,1}; x bf16 rounding gives
    ~2e-3 rel error on the output. PSUM accumulates in f32.
  - Output tiles [co=128, 448] (8 rows of 56) get bias added on VectorE on
    the way out (PSUM -> SBUF f32), then DMA to DRAM.
"""

import sys

sys.path.insert(0, "/opt/trn_rl_repo")

from contextlib import ExitStack

import numpy as np

B, CI, CO, KK, H, W = 32, 128, 256, 3, 56, 56
N_CORES = 8
B_SH = B // N_CORES  # 4 images per core
HP, WP = H + 2, W + 2  # padded 58x58
ROWS_PER_MM = 8  # output rows per matmul -> N = 448 <= 512 (one PSUM bank)
N_MM = ROWS_PER_MM * W  # 448
N_RB = H // ROWS_PER_MM  # 7 row blocks

_NC_CACHE = None


def _build():
    import concourse.tile as tile
    from concourse import bacc, mybir

    nc = bacc.Bacc("TRN2", target_bir_lowering=False, debug=False)

    x_d = nc.dram_tensor("x", [B_SH, CI, H, W], mybir.dt.float32, kind="ExternalInput")
    wt_d = nc.dram_tensor(
        "wt", [CI, KK * KK * CO], mybir.dt.float32, kind="ExternalInput"
    )
    b_d = nc.dram_tensor("bias2", [128, CO // 128], mybir.dt.float32, kind="ExternalInput")
    y_d = nc.dram_tensor("y", [B_SH, CO, H, W], mybir.dt.float32, kind="ExternalOutput")

    x_ap = x_d.ap().rearrange("b c h w -> b c (h w)")
    y_ap = y_d.ap().rearrange("b c h w -> b c (h w)")

    with tile.TileContext(nc) as tc:
        with ExitStack() as ctx:
            singles = ctx.enter_context(tc.tile_pool(name="singles", bufs=1))
            xf_pool = ctx.enter_context(tc.tile_pool(name="xf", bufs=2))
            xp_pool = ctx.enter_context(tc.tile_pool(name="xp", bufs=2))
            ps_pool = ctx.enter_context(
                tc.tile_pool(name="ps", bufs=6, space="PSUM")
            )
            yo_pool = ctx.enter_context(tc.tile_pool(name="yo", bufs=4))

            # --- weight prep: DMA f32 [ci, 9*co], sign -> bf16 ---
            w_f32 = singles.tile([CI, KK * KK * CO], mybir.dt.float32)
            nc.sync.dma_start(out=w_f32[:, :], in_=wt_d.ap())
            w_bin = singles.tile([CI, KK * KK * CO], mybir.dt.bfloat16)
            nc.scalar.sign(w_bin[:, :], w_f32[:, :])
            w_bin3 = w_bin.rearrange("p (t c) -> p t c", c=CO)

            bias_sb = singles.tile([128, CO // 128], mybir.dt.float32)
            nc.sync.dma_start(out=bias_sb[:, :], in_=b_d.ap())

            for b in range(B_SH):
                xf = xf_pool.tile([CI, H * W], mybir.dt.float32)
                nc.sync.dma_start(out=xf[:, :], in_=x_ap[b])
                xf3 = xf.rearrange("p (h w) -> p h w", w=W)

                xp = xp_pool.tile([CI, HP * WP], mybir.dt.bfloat16)
                xp3 = xp.rearrange("p (h w) -> p h w", w=WP)
                # zero the 1-wide border (interior is fully overwritten)
                nc.vector.memset(xp3[:, 0, :], 0.0)
                nc.vector.memset(xp3[:, HP - 1, :], 0.0)
                nc.vector.memset(xp3[:, 1 : HP - 1, 0], 0.0)
                nc.vector.memset(xp3[:, 1 : HP - 1, WP - 1], 0.0)
                # interior: cast f32 -> bf16
                nc.vector.tensor_copy(
                    out=xp3[:, 1 : HP - 1, 1 : WP - 1], in_=xf3[:, :, :]
                )

                for c2 in range(CO // 128):
                    for rb in range(N_RB):
                        ps = ps_pool.tile([128, N_MM], mybir.dt.float32)
                        r0 = rb * ROWS_PER_MM
                        i = 0
                        for kh in range(KK):
                            for kw in range(KK):
                                rhs = xp3[:, r0 + kh : r0 + kh + ROWS_PER_MM, kw : kw + W]
                                lhsT = w_bin3[:, kh * KK + kw, c2 * 128 : (c2 + 1) * 128]
                                nc.tensor.matmul(
                                    ps[:, :],
                                    lhsT,
                                    rhs,
                                    start=(i == 0),
                                    stop=(i == KK * KK - 1),
                                )
                                i += 1
                        ys = yo_pool.tile([128, N_MM], mybir.dt.float32)
                        nc.vector.tensor_scalar_add(
                            ys[:, :], ps[:, :], bias_sb[:, c2 : c2 + 1]
                        )
                        nc.sync.dma_start(
                            out=y_ap[
                                b,
                                c2 * 128 : (c2 + 1) * 128,
                                rb * N_MM : (rb + 1) * N_MM,
                            ],
                            in_=ys[:, :],
                        )
    nc.compile()
    return nc


def _get_nc():
    global _NC_CACHE
    if _NC_CACHE is None:
        _NC_CACHE = _build()
    return _NC_CACHE


def kernel(x, weight, bias):
    from concourse.bass_utils import run_bass_kernel_spmd

    x = np.ascontiguousarray(np.asarray(x, dtype=np.float32))
    weight = np.asarray(weight, dtype=np.float32)
    bias = np.asarray(bias, dtype=np.float32)

    # [co, ci, kh, kw] -> [ci, (kh kw co)]
    wt = np.ascontiguousarray(
        weight.transpose(1, 2, 3, 0).reshape(CI, KK * KK * CO)
    )
    # bias2[p, c2] = bias[c2*128 + p]
    bias2 = np.ascontiguousarray(bias.reshape(CO // 128, 128).T)

    nc = _get_nc()
    in_maps = [
        {"x": x[i * B_SH : (i + 1) * B_SH], "wt": wt, "bias2": bias2}
        for i in range(N_CORES)
    ]
    res = run_bass_kernel_spmd(nc, in_maps, core_ids=list(range(N_CORES)))
    return np.concatenate([r["y"] for r in res.results], axis=0)


# revision 4
# speedup vs baseline: 1.1247x; 1.1247x over previous
"""Binary (sign-quantized weight) 3x3 conv, stride 1, pad 1, on 8 trn2 cores.

Problem: x[32,128,56,56] f32, weight[256,128,3,3] f32, bias[256] f32
         y = conv2d(x, sign(weight), pad=1) + bias      -> [32,256,56,56] f32

Strategy:
  - Data-parallel over batch: 4 images per core, weight/bias replicated.
  - Per core: x is loaded per-image as [ci=128 partitions, 56*56], cast to
    bf16 into a zero-padded [128, 58*58] tile. The 3x3 conv is 9 shifted
    [128ci -> 128co] matmuls accumulated in PSUM (implicit im2col via
    strided access patterns into the padded tile).
  - Weights are host-relaid to [ci, (kh kw co)] f32; sign+cast to bf16 on
    device (ScalarE). bf16 is exact for {-1,0,1}; x bf16 rounding gives
    ~2e-3 rel error on the output. PSUM accumulates in f32.
  - Output tiles [co=128, 448] (8 rows of 56) get bias added on VectorE on
    the way out (PSUM -> SBUF f32), then DMA to DRAM.
"""

import sys

sys.path.insert(0, "/opt/trn_rl_repo")

from contextlib import ExitStack

import numpy as np

B, CI, CO, KK, H, W = 32, 128, 256, 3, 56, 56
N_CORES = 8
B_SH = B // N_CORES  # 4 images per core
HP, WP = H + 2, W + 2  # padded 58x58
ROWS_PER_MM = 8  # output rows per matmul -> N = 448 <= 512 (one PSUM bank)
N_MM = ROWS_PER_MM * W  # 448
N_RB = H // ROWS_PER_MM  # 7 row blocks

_NC_CACHE = None


def _build():
    import concourse.tile as tile
    from concourse import bacc, mybir

    nc = bacc.Bacc("TRN2", target_bir_lowering=False, debug=False)

    x_d = nc.dram_tensor("x", [B_SH, CI, H, W], mybir.dt.float32, kind="ExternalInput")
    wt_d = nc.dram_tensor(
        "wt", [CI, KK * KK * CO], mybir.dt.float32, kind="ExternalInput"
    )
    b_d = nc.dram_tensor("bias2", [128, CO // 128], mybir.dt.float32, kind="ExternalInput")
    y_d = nc.dram_tensor("y", [B_SH, CO, H, W], mybir.dt.float32, kind="ExternalOutput")

    x_ap = x_d.ap().rearrange("b c h w -> b c (h w)")
    y_ap = y_d.ap().rearrange("b c h w -> b c (h w)")

    with tile.TileContext(nc) as tc:
        with ExitStack() as ctx:
            singles = ctx.enter_context(tc.tile_pool(name="singles", bufs=1))
            xf_pool = ctx.enter_context(tc.tile_pool(name="xf", bufs=2))
            xp_pool = ctx.enter_context(tc.tile_pool(name="xp", bufs=2))
            ps_pool = ctx.enter_context(
                tc.tile_pool(name="ps", bufs=6, space="PSUM")
            )
            yo_pool = ctx.enter_context(tc.tile_pool(name="yo", bufs=4))

            # --- weight prep: per-tap DMA f32 [ci, co] + sign -> bf16, so the
            # first matmul's weights are ready ASAP ---
            wt_ap = wt_d.ap().rearrange("p (t c) -> p t c", c=CO)
            w_f32 = singles.tile([CI, KK * KK, CO], mybir.dt.float32)
            w_bin = singles.tile([CI, KK * KK, CO], mybir.dt.bfloat16)
            w_bin3 = w_bin

            x_img = x_ap.rearrange("b c (h w) -> b c h w", w=W)

            def load_chunk(xf3, xp3, b, c):
                r0 = c * ROWS_PER_MM
                nc.sync.dma_start(
                    out=xf3[:, r0 : r0 + ROWS_PER_MM, :],
                    in_=x_img[b, :, r0 : r0 + ROWS_PER_MM, :],
                )
                nc.vector.tensor_copy(
                    out=xp3[:, 1 + r0 : 1 + r0 + ROWS_PER_MM, 1 : WP - 1],
                    in_=xf3[:, r0 : r0 + ROWS_PER_MM, :],
                )

            def alloc_img(b):
                xf = xf_pool.tile([CI, H * W], mybir.dt.float32)
                xf3 = xf.rearrange("p (h w) -> p h w", w=W)
                xp = xp_pool.tile([CI, HP * WP], mybir.dt.bfloat16)
                xp3 = xp.rearrange("p (h w) -> p h w", w=WP)
                # zero the 1-wide border (interior is fully overwritten)
                nc.vector.memset(xp3[:, 0, :], 0.0)
                nc.vector.memset(xp3[:, HP - 1, :], 0.0)
                nc.vector.memset(xp3[:, 1 : HP - 1, 0], 0.0)
                nc.vector.memset(xp3[:, 1 : HP - 1, WP - 1], 0.0)
                return xf3, xp3

            def load_tap(t):
                nc.sync.dma_start(out=w_f32[:, t, :], in_=wt_ap[:, t, :])
                nc.scalar.sign(w_bin[:, t, :], w_f32[:, t, :])

            # startup-critical order: b=0 chunks 0/1 and tap 0 first
            xf3_0, xp3_0 = alloc_img(0)
            load_chunk(xf3_0, xp3_0, 0, 0)
            load_tap(0)
            load_chunk(xf3_0, xp3_0, 0, 1)
            for t in range(1, KK * KK):
                load_tap(t)
            bias_sb = singles.tile([128, CO // 128], mybir.dt.float32)
            nc.sync.dma_start(out=bias_sb[:, :], in_=b_d.ap())
            for c in range(2, N_RB):
                load_chunk(xf3_0, xp3_0, 0, c)

            for b in range(B_SH):
                if b == 0:
                    xp3 = xp3_0
                else:
                    xf3, xp3 = alloc_img(b)
                    for c in range(N_RB):
                        load_chunk(xf3, xp3, b, c)

                for c2 in range(CO // 128):
                    for rb in range(N_RB):
                        ps = ps_pool.tile([128, N_MM], mybir.dt.float32)
                        r0 = rb * ROWS_PER_MM
                        i = 0
                        for kh in range(KK):
                            for kw in range(KK):
                                rhs = xp3[:, r0 + kh : r0 + kh + ROWS_PER_MM, kw : kw + W]
                                lhsT = w_bin3[:, kh * KK + kw, c2 * 128 : (c2 + 1) * 128]
                                nc.tensor.matmul(
                                    ps[:, :],
                                    lhsT,
                                    rhs,
                                    start=(i == 0),
                                    stop=(i == KK * KK - 1),
                                )
                                i += 1
                        ys = yo_pool.tile([128, N_MM], mybir.dt.float32)
                        nc.vector.tensor_scalar_add(
                            ys[:, :], ps[:, :], bias_sb[:, c2 : c2 + 1]
                        )
                        nc.sync.dma_start(
                            out=y_ap[
                                b,
                                c2 * 128 : (c2 + 1) * 128,
                                rb * N_MM : (rb + 1) * N_MM,
                            ],
                            in_=ys[:, :],
                        )
    nc.compile()
    return nc


def _get_nc():
    global _NC_CACHE
    if _NC_CACHE is None:
        _NC_CACHE = _build()
    return _NC_CACHE


def kernel(x, weight, bias):
    from concourse.bass_utils import run_bass_kernel_spmd

    x = np.ascontiguousarray(np.asarray(x, dtype=np.float32))
    weight = np.asarray(weight, dtype=np.float32)
    bias = np.asarray(bias, dtype=np.float32)

    # [co, ci, kh, kw] -> [ci, (kh kw co)]
    wt = np.ascontiguousarray(
        weight.transpose(1, 2, 3, 0).reshape(CI, KK * KK * CO)
    )
    # bias2[p, c2] = bias[c2*128 + p]
    bias2 = np.ascontiguousarray(bias.reshape(CO // 128, 128).T)

    nc = _get_nc()
    in_maps = [
        {"x": x[i * B_SH : (i + 1) * B_SH], "wt": wt, "bias2": bias2}
        for i in range(N_CORES)
    ]
    res = run_bass_kernel_spmd(nc, in_maps, core_ids=list(range(N_CORES)))
    return np.concatenate([r["y"] for r in res.results], axis=0)


# revision 10
# speedup vs baseline: 1.1761x; 1.0456x over previous
"""Binary (sign-quantized weight) 3x3 conv, stride 1, pad 1, on 8 trn2 cores.

Problem: x[32,128,56,56] f32, weight[256,128,3,3] f32, bias[256] f32
         y = conv2d(x, sign(weight), pad=1) + bias      -> [32,256,56,56] f32

Strategy:
  - Data-parallel over batch: 4 images per core, weight/bias replicated.
  - Per core: x is loaded per-image as [ci=128 partitions, 56*56] f32 and
    cast to bf16 (unit stride, no physical padding). The 3x3 conv is 9
    shifted [128ci -> 128co] matmuls accumulated in PSUM per output tile
    of 8 rows x 56 cols (N=448). Padding is implicit: boundary taps use
    narrowed row/col ranges (PSUM per-element has_written gives
    overwrite-on-first-write, so partial-coverage accumulation is exact).
  - Weights are host-relaid to [ci, (kh kw co)] f32; sign+cast to bf16 on
    device (ScalarE). bf16 is exact for {-1,0,1}; x bf16 rounding gives
    ~2e-3 rel error. PSUM accumulates in f32.
  - Output tiles [co=128, 448] get bias added on VectorE on the way out
    (PSUM -> SBUF f32), then DMA to DRAM.
  - Warm-up: dummy sign op preloads the ACT table; zero matmuls keep the
    PE busy from t~0 so the HAM clock gate is at full speed when real
    matmuls start.
"""

import sys

sys.path.insert(0, "/opt/trn_rl_repo")

from contextlib import ExitStack

import numpy as np

B, CI, CO, KK, H, W = 32, 128, 256, 3, 56, 56
N_CORES = 8
B_SH = B // N_CORES  # 4 images per core
ROWS_PER_MM = 8  # output rows per matmul -> N = 448 <= 512 (one PSUM bank)
N_MM = ROWS_PER_MM * W  # 448
N_RB = H // ROWS_PER_MM  # 7 row blocks

_NC_CACHE = None


def _build():
    import concourse.tile as tile
    from concourse import bacc, mybir

    nc = bacc.Bacc("TRN2", target_bir_lowering=False, debug=False)

    x_d = nc.dram_tensor("x", [B_SH, CI, H, W], mybir.dt.float32, kind="ExternalInput")
    wt_d = nc.dram_tensor(
        "wt", [CI, KK * KK * CO], mybir.dt.float32, kind="ExternalInput"
    )
    b_d = nc.dram_tensor("bias2", [128, CO // 128], mybir.dt.float32, kind="ExternalInput")
    y_d = nc.dram_tensor("y", [B_SH, CO, H, W], mybir.dt.float32, kind="ExternalOutput")

    x_ap = x_d.ap().rearrange("b c h w -> b c (h w)")
    y_ap = y_d.ap().rearrange("b c h w -> b c (h w)")
    x_img = x_d.ap()  # [b, c, h, w]

    with tile.TileContext(nc) as tc:
        with ExitStack() as ctx:
            singles = ctx.enter_context(tc.tile_pool(name="singles", bufs=1))
            xf_pool = ctx.enter_context(tc.tile_pool(name="xf", bufs=3))
            xb_pool = ctx.enter_context(tc.tile_pool(name="xb", bufs=3))
            ps_pool = ctx.enter_context(
                tc.tile_pool(name="ps", bufs=8, space="PSUM")
            )
            yo_pool = ctx.enter_context(tc.tile_pool(name="yo", bufs=6))

            wt_ap = wt_d.ap().rearrange("p (t c) -> p t c", c=CO)
            w_f32 = singles.tile([CI, KK * KK, CO], mybir.dt.float32)
            w_bin = singles.tile([CI, KK * KK, CO], mybir.dt.bfloat16)

            # ACT warm-up: charge the Sign activation-table load (~1.3us)
            # while the first DMAs are still in flight
            warm = singles.tile([128, 1], mybir.dt.float32)
            nc.vector.memset(warm[:, :], 0.0)
            nc.scalar.sign(warm[:, :], warm[:, :])

            # PE warm-up: zero matmuls so the HAM clock gate (and the cost
            # model's p-state ramp) is at full speed when real matmuls begin
            warm_w = singles.tile([128, 128], mybir.dt.bfloat16)
            warm_x = singles.tile([128, N_MM], mybir.dt.bfloat16)
            nc.vector.memset(warm_w[:, :], 0.0)
            nc.vector.memset(warm_x[:, :], 0.0)
            for _ in range(8):
                warm_ps = ps_pool.tile([128, N_MM], mybir.dt.float32, tag="ps")
                nc.tensor.matmul(
                    warm_ps[:, :], warm_w[:, :], warm_x[:, :], start=True, stop=True
                )

            def load_chunk(xf3, xb3, b, c):
                r0 = c * ROWS_PER_MM
                nc.sync.dma_start(
                    out=xf3[:, r0 : r0 + ROWS_PER_MM, :],
                    in_=x_img[b, :, r0 : r0 + ROWS_PER_MM, :],
                )
                nc.vector.tensor_copy(
                    out=xb3[:, r0 : r0 + ROWS_PER_MM, :],
                    in_=xf3[:, r0 : r0 + ROWS_PER_MM, :],
                )

            def alloc_img():
                xf = xf_pool.tile([CI, H * W], mybir.dt.float32, tag="xf")
                xb = xb_pool.tile([CI, H * W], mybir.dt.bfloat16, tag="xb")
                return (
                    xf.rearrange("p (h w) -> p h w", w=W),
                    xb.rearrange("p (h w) -> p h w", w=W),
                )

            def load_tap(t):
                nc.sync.dma_start(out=w_f32[:, t, :], in_=wt_ap[:, t, :])
                nc.scalar.sign(w_bin[:, t, :], w_f32[:, t, :])

            # startup-critical order: b=0 chunks 0/1 and tap 0 first
            xf3_0, xb3_0 = alloc_img()
            load_chunk(xf3_0, xb3_0, 0, 0)
            load_tap(0)
            load_chunk(xf3_0, xb3_0, 0, 1)
            load_tap(1)
            load_tap(2)
            load_chunk(xf3_0, xb3_0, 0, 2)
            load_tap(3)
            load_tap(4)
            load_chunk(xf3_0, xb3_0, 0, 3)
            load_tap(5)
            load_tap(6)
            load_chunk(xf3_0, xb3_0, 0, 4)
            load_tap(7)
            load_tap(8)
            bias_sb = singles.tile([128, CO // 128], mybir.dt.float32)
            nc.sync.dma_start(out=bias_sb[:, :], in_=b_d.ap())
            for c in range(5, N_RB):
                load_chunk(xf3_0, xb3_0, 0, c)

            for b in range(B_SH):
                if b == 0:
                    xb3 = xb3_0
                else:
                    xf3, xb3 = alloc_img()
                    for c in range(N_RB):
                        load_chunk(xf3, xb3, b, c)

                for c2 in range(CO // 128):
                    for rb in range(N_RB):
                        r0 = rb * ROWS_PER_MM
                        ps = ps_pool.tile([128, N_MM], mybir.dt.float32, tag="ps")
                        ps3 = ps.rearrange("p (r w) -> p r w", w=W)
                        i = 0
                        for kh in range(KK):
                            # output rows (within block) whose input row is
                            # in [0, H)
                            a = max(0, (1 - kh) - r0)
                            bb = min(ROWS_PER_MM, (H + 1) - kh - r0)
                            for kw in range(KK):
                                c0 = max(0, 1 - kw)
                                c1 = W - max(0, kw - 1)
                                rhs = xb3[
                                    :,
                                    r0 + a + kh - 1 : r0 + bb + kh - 1,
                                    c0 + kw - 1 : c1 + kw - 1,
                                ]
                                lhsT = w_bin[:, kh * KK + kw, c2 * 128 : (c2 + 1) * 128]
                                nc.tensor.matmul(
                                    ps3[:, a:bb, c0:c1],
                                    lhsT,
                                    rhs,
                                    start=(i == 0),
                                    stop=(i == KK * KK - 1),
                                    skip_group_check=True,
                                )
                                i += 1
                        ys = yo_pool.tile([128, N_MM], mybir.dt.float32, tag="ys")
                        nc.vector.tensor_scalar_add(
                            ys[:, :], ps[:, :], bias_sb[:, c2 : c2 + 1]
                        )
                        nc.sync.dma_start(
                            out=y_ap[
                                b,
                                c2 * 128 : (c2 + 1) * 128,
                                rb * N_MM : (rb + 1) * N_MM,
                            ],
                            in_=ys[:, :],
                        )
    nc.compile()
    return nc


def _get_nc():
    global _NC_CACHE
    if _NC_CACHE is None:
        _NC_CACHE = _build()
    return _NC_CACHE


def kernel(x, weight, bias):
    from concourse.bass_utils import run_bass_kernel_spmd

    x = np.ascontiguousarray(np.asarray(x, dtype=np.float32))
    weight = np.asarray(weight, dtype=np.float32)
    bias = np.asarray(bias, dtype=np.float32)

    # [co, ci, kh, kw] -> [ci, (kh kw co)]
    wt = np.ascontiguousarray(
        weight.transpose(1, 2, 3, 0).reshape(CI, KK * KK * CO)
    )
    # bias2[p, c2] = bias[c2*128 + p]
    bias2 = np.ascontiguousarray(bias.reshape(CO // 128, 128).T)

    nc = _get_nc()
    in_maps = [
        {"x": x[i * B_SH : (i + 1) * B_SH], "wt": wt, "bias2": bias2}
        for i in range(N_CORES)
    ]
    res = run_bass_kernel_spmd(nc, in_maps, core_ids=list(range(N_CORES)))
    return np.concatenate([r["y"] for r in res.results], axis=0)
